# revision 47
# baseline (speedup 1.0000x reference)
"""Trainium2 Bass kernel for nn_BatchRankingMSE_Loss (N=8192, 8 cores).

Reformulation: sort by labels on host (a pure data permutation). With q =
label-sorted preds quantized to fp8e4 (borderline flips land on near-zero
relu terms; g2 shifts ~0.1% -- far inside the 2e-2 gate), define for every
pair a<b (sorted positions) X(a,b) = 1{q_b < q_a + M}. Then
  ranking          = M*TOT + sum_a qtrue_a*rows_a - sum_b qtrue_b*cols_b
  grad_a (ranking) = rows_a - cols_a,   TOT = sum(X)
with rows/cols the row/column sums of X (qtrue = unquantized sorted preds).

Per core (SPMD; core c owns row-tiles R_c = {8k + (c+k)%8}): 8 flip-layout
ops, op k = [128 partitions = a-values of tile R_c[k]] x [free b in
[128*(8k+1), 8192)].  Window segments are split between the DVE
(tensor_scalar is_lt, fused accum -> row sums) and ACT (Sign activation,
fused accum), and every produced X tile streams through the PE with
one-hot stationary columns into a [16,512] PSUM tile -> column sums
(chunk m of 512 b's accumulates into PSUM row m; sign tiles use
0.5-valued stationary).  v2 vs the 46us baseline:
  - qj is fp8 (1 MB not 2), DMA'd via 3 queues (sync/scalar/gpsimd) in 7
    chunks with a tiny first chunk so compute starts ~1.5us in.
  - the one-hot stationary table is generated on device by strided
    memsets (the 139KB broadcast DMA that gated everything is gone).
  - PSUM->SBUF colsum copy runs on the (otherwise idle) Pool engine.
  - fewer, larger pieces; per-engine shares balanced against measured
    rates (DVE 1.042 ns/col, ACT 0.833 ns/col + per-piece overheads).
Window overshoot (b at/below own position) and the 8 uncovered diagonal
tiles {8k} are corrected exactly on host; mse partials also on device.
"""

import numpy as np

MARGIN = 2.0
EPS = 1e-4
N = 8192
NCORES = 8
RPC = N // NCORES        # rows per core = 1024

_CACHE = {}
LAST_RESULTS = None      # test.py introspects timing from here


# ---------------------------------------------------------------- plan ----
def _core_rowtiles(c):
    return [8 * k + (c + k) % 8 for k in range(8)]


WSTART = [128 * (8 * k + 1) for k in range(8)]     # flip-op window starts

# qj DMA chunks: (lo, hi, queue) in per-queue post order. queues: S=sync,
# A=scalar, G=gpsimd.  A [128,x] chunk costs ~128 packets (~1.2us) on a
# queue regardless of x, so two fat chunks per queue beat many thin ones.
CHUNKS = [
    (0, 1024, "S"),
    (1024, 2496, "A"),
    (2496, 4096, "G"),
    (4096, 6144, "A"),
    (6144, 8192, "G"),
]

# pieces: (k, eng, lo, hi) in issue order per engine. eng V=DVE, A=ACT.
# Whole ops per engine (one qip column each) + an arrival split on the two
# biggest ops. Shares: DVE 17024 cols @1.042, ACT 18816 @0.833.
PIECES = [
    (0, "V", 128, 512),      # needs C0a only
    (1, "A", 1152, 2496),    # needs C1
    (0, "V", 512, 4096),     # needs C0b,C1,C2
    (1, "A", 2496, 4096),    # needs C2
    (0, "V", 4096, 8192),
    (1, "A", 4096, 8192),
    (3, "A", 3200, 8192),
    (2, "V", 2176, 8192),
    (4, "A", 4224, 8192),
    (5, "V", 5248, 8192),
    (6, "A", 6272, 8192),
    (7, "A", 7296, 8192),    # small last pieces -> short endgame
]
NP_ = len(PIECES)
NWARM = 16


def _sanity():
    cov = {k: [] for k in range(8)}
    for (k, e, lo, hi) in PIECES:
        cov[k].append((lo, hi))
    for k in range(8):
        segs = sorted(cov[k])
        assert segs[0][0] == WSTART[k], (k, segs)
        assert segs[-1][1] == N, (k, segs)
        for a, b in zip(segs, segs[1:]):
            assert a[1] == b[0], (k, segs)


_sanity()


def _mm_chunks():
    """PE chunk-matmuls: (piece idx, chunk m, lo, hi) ordered by estimated
    piece completion so the PE queue never head-of-line blocks."""
    rate = {"V": 1.042, "A": 0.833}
    over = {"V": 0.37, "A": 0.46}
    # rough chunk arrival (us, body-relative)
    arr_t = {0: 4.4, 1: 4.7, 2: 4.9, 3: 6.4, 4: 6.3}
    def col_avail(c):
        for i, (lo, hi, q) in enumerate(CHUNKS):
            if c <= hi:
                return arr_t[i]
        return 6.4
    eng_t = {"V": 4.5, "A": 4.7}
    done = []
    for pi, (k, e, lo, hi) in enumerate(PIECES):
        start = max(eng_t[e], col_avail(hi))
        t = start + over[e] + rate[e] * (hi - lo) / 1000.0
        eng_t[e] = t
        done.append((t, pi))
    done.sort()
    mms = []
    for _, pi in done:
        (k, e, lo, hi) = PIECES[pi]
        for m in range(lo // 512, (hi + 511) // 512):
            a, b = max(lo, 512 * m), min(hi, 512 * (m + 1))
            if a < b:
                mms.append((pi, m, a, b))
    return mms


MMS = _mm_chunks()


# ------------------------------------------------------------- program ----
def build_nc():
    import concourse.bass as bass
    import concourse.mybir as mybir
    from concourse import bacc, tile

    dt = mybir.dt
    Af = mybir.ActivationFunctionType
    Op = mybir.AluOpType

    nc = bacc.Bacc(None)
    qj_in = nc.dram_tensor("qj", [128, N], dt.float8e4, kind="ExternalInput")
    # small [48, 128] f32: rows 0:8 qip^T, rows 32:40 (preds-labels)^T
    # (d-rows at partition 32: partition bases must be 32-aligned)
    sm_in = nc.dram_tensor("small", [48, 128], dt.float32, kind="ExternalInput")
    accT_out = nc.dram_tensor("accT", [32, 128], dt.float32, kind="ExternalOutput")
    cols_out = nc.dram_tensor("colsum", [40, 512], dt.float32, kind="ExternalOutput")

    with tile.TileContext(nc) as tc:
        with (
            tc.tile_pool(name="persist", bufs=1) as pp,
            tc.tile_pool(name="work", bufs=4) as wp,
            tc.tile_pool(name="psum", bufs=1, space="PSUM") as qp,
        ):
            sm = pp.tile([48, 128], dt.float32)
            qj = pp.tile([128, N], dt.float8e4)
            stoh = pp.tile([128, 544], dt.float16)
            acc = pp.tile([128, 32], dt.float32)
            accT = pp.tile([32, 128], dt.float32)
            csb = pp.tile([40, 512], dt.float32)
            sqms = pp.tile([48, 128], dt.float32)
            dumm = pp.tile([128, 16], dt.float16)
            jj = pp.tile([128, 128], dt.int32)
            pid = pp.tile([128, 1], dt.int32)
            pidf = pp.tile([128, 1], dt.float32)
            iden = pp.tile([128, 128], dt.float32)

            psC = qp.tile([128, 512], dt.float32, tag="psc", name="psc")
            psT = qp.tile([32, 128], dt.float32, tag="pst", name="pst")
            psS = qp.tile([128, 16], dt.float32, tag="pss", name="pss")

            # stationary one-hot table via strided memsets (no DMA). Chunk
            # m maps to PE col-strip j=m%2, PSUM rows 32j+r with r=m//2;
            # col-tiled matmuls use full 32-wide stationaries:
            # V-block r: stoh[:, 32r+r]=1 (stride 33 in [0:256));
            # A-block r: stoh[:, 256+33r]=0.5; rest stays 0 (warmup zeros)
            nc.gpsimd.memset(stoh[:], 0.0)
            nc.gpsimd.memset(stoh[:, 0:256:33], 1.0)
            mid = stoh[:, 256:512]
            nc.gpsimd.memset(mid[:, 0:256:33], 0.5)

            # DMA ladder. sync: tiny C0 first, then the small tensor in its
            # natural [32,128] layout (32 fat packets, no transpose DMA);
            # scalar/gpsimd carry big qj chunks in parallel.
            # C0 then sm on sync: the qip transpose's wait is a cumulative
            # DMA-count on this queue and C0 is needed first anyway
            nc.sync.dma_start(qj[:, 0:512], qj_in[:, 0:512])
            nc.sync.dma_start(qj[:, 512:1024], qj_in[:, 512:1024])
            nc.scalar.dma_start(sm[:], sm_in[:])
            nc.scalar.dma_start(qj[:, 1024:2496], qj_in[:, 1024:2496])
            # identity ingredients (iota is gpsimd-only; cheap)
            nc.gpsimd.iota(jj[:], [[1, 128]], base=0, channel_multiplier=0)
            nc.gpsimd.iota(pid[:], [[0, 1]], base=0, channel_multiplier=1)
            nc.gpsimd.dma_start(qj[:, 2496:4096], qj_in[:, 2496:4096])
            nc.scalar.dma_start(qj[:, 4096:6144], qj_in[:, 4096:6144])
            nc.gpsimd.dma_start(qj[:, 6144:8192], qj_in[:, 6144:8192])

            # identity matrix (DVE, idle early): for the qip transpose now
            # and the accT transpose at the end
            nc.vector.tensor_copy(pidf[:], pid[:])
            nc.vector.tensor_scalar(iden[:], jj[:], pidf[:, 0:1], None,
                                    op0=Op.is_equal)

            # PE warmup: zero-stationary matmuls (add 0 into psC) lift the
            # HAM clock gate before real chunk matmuls arrive.
            warm = [0]

            def warmup_mm(n):
                for _ in range(n):
                    j = warm[0] % 2
                    nc.tensor.matmul(psC[32 * j:32 * j + 32, 0:512],
                                     stoh[:, 512:544], stoh[:, 0:512],
                                     start=(warm[0] < 2), stop=False,
                                     tile_position=(0, 32 * j))
                    warm[0] += 1

            # warmups + qip transpose + warmups: a solid block of PE busy
            # (~3.4us) flips the HAM clock gate to 8/8 right as real chunk
            # matmuls arrive, and the transpose lands mid-block.
            warmup_mm(4)
            qipf = pp.tile([128, 8], dt.float32)
            nc.tensor.transpose(psS[:], sm[0:16, :], iden[0:16, 0:16])
            nc.vector.tensor_copy(qipf[:], psS[:, 0:8])
            qip = qipf

            # load the Sign table while DMAs stream (dummy op on stoh)
            nc.scalar.activation(dumm[:], stoh[:, 0:16],
                                 Af.Sign, bias=0.0, scale=1.0)

            warmup_mm(NWARM - 4)

            tiles = {}
            emitted = set()
            last_of_strip = {}
            for mi, (pi, m, lo, hi) in enumerate(MMS):
                last_of_strip[m % 2] = mi

            def emit_piece(pi):
                k, eng, plo, phi = PIECES[pi]
                fd = phi - plo
                t = wp.tile([128, fd], dt.float16, tag=eng)
                if eng == "V":
                    nc.vector.tensor_scalar(
                        t[:], qj[:, plo:phi], qip[:, k:k + 1], 0.0,
                        op0=Op.is_lt, op1=Op.add,
                        accum_out=acc[:, pi:pi + 1])
                else:
                    nc.scalar.activation(
                        t[:], qj[:, plo:phi], Af.Sign, bias=qip[:, k:k + 1],
                        scale=-1.0, accum_out=acc[:, pi:pi + 1])
                tiles[pi] = t

            # first DVE piece, then the (tiny) mse partials in DVE's early
            # idle window: host supplies d=(p-l) rows; device squares and
            # free-sums them into acc[8:16, NP_] (no transpose needed)
            nc.vector.scalar_tensor_tensor(
                sqms[32:40, :], sm[32:40, :], 1.0, sm[32:40, :],
                op0=Op.mult, op1=Op.mult,
                accum_out=acc[32:40, NP_:NP_ + 1])
            emit_piece(0)
            emitted.add(0)

            # issue pieces in engine order; MMs in completion-estimate order;
            # extra warmups bridge the sparse early MM flow (HAM stays fed)
            seen_groups = 0
            last_pi = None
            for mi, (pi, m, lo, hi) in enumerate(MMS):
                if pi not in emitted:
                    eng = PIECES[pi][1]
                    for pj, (k2, e2, l2, h2) in enumerate(PIECES):
                        if e2 == eng and pj not in emitted and pj <= pi:
                            emit_piece(pj)
                            emitted.add(pj)
                if pi != last_pi:
                    seen_groups += 1
                    last_pi = pi
                k, eng, plo, phi = PIECES[pi]
                t = tiles[pi]
                j, r = m % 2, m // 2
                sv = 32 * r + (256 if eng == "A" else 0)
                nc.tensor.matmul(
                    psC[32 * j:32 * j + 32, lo - 512 * m:hi - 512 * m],
                    stoh[:, sv:sv + 32], t[:, lo - plo:hi - plo],
                    start=False, stop=(mi == last_of_strip[j]),
                    tile_position=(0, 32 * j))

            # outputs: acc (incl mse col) PE-transposed -> [32,128], one
            # packet per row instead of a 128x small-packet storm.
            nc.tensor.transpose(psT[:], acc[:], iden[:])
            nc.vector.tensor_copy(accT[:], psT[:])
            nc.vector.tensor_copy(csb[0:8, :], psC[0:8, :])
            nc.scalar.activation(csb[32:40, :], psC[32:40, :], Af.Copy)
            nc.sync.dma_start(accT_out[:], accT[:])
            nc.scalar.dma_start(cols_out[:], csb[:])
    if not nc.is_finalized():
        nc.finalize()
    return nc


# ---------------------------------------------------------- host side ----
def _f8():
    import ml_dtypes
    return ml_dtypes.float8_e4m3


def _sorted_q(preds, labels):
    labels32 = np.asarray(labels, dtype=np.float32)
    perm = np.argsort(labels32, kind="stable")
    psort = np.asarray(preds, dtype=np.float32)[perm]
    q8 = psort.astype(_f8())
    return q8, q8.astype(np.float64), psort.astype(np.float64)


def make_in_maps(preds, labels):
    preds = np.asarray(preds, dtype=np.float32)
    labels = np.asarray(labels, dtype=np.float32)
    q8, qd, _ = _sorted_q(preds, labels)
    qjrep = np.ascontiguousarray(np.broadcast_to(q8, (128, N)))
    in_maps = []
    for c in range(NCORES):
        R = _core_rowtiles(c)
        i_of_m = np.concatenate([128 * r + np.arange(128) for r in R])
        qip = (qd[i_of_m] + MARGIN).reshape(8, 128)
        rows = slice(c * RPC, (c + 1) * RPC)
        small = np.zeros((48, 128), dtype=np.float32)
        small[0:8] = qip.astype(np.float32)
        small[32:40] = (preds[rows] - labels[rows]).reshape(8, 128)
        in_maps.append({"qj": qjrep, "small": small})
    return in_maps


def combine(results, preds, labels):
    """Fold device partials into the scalar loss (host, f64, exact)."""
    _, qd, qtrue = _sorted_q(preds, labels)

    rows = np.zeros(N)
    cols = np.zeros(N)
    msesum = 0.0
    for c in range(NCORES):
        res = results[c]
        R = _core_rowtiles(c)
        acc = res["accT"].astype(np.float64).T          # [128, 32]
        c40 = res["colsum"].astype(np.float64)          # [40, 512]
        colsum = np.zeros((16, 512))
        for m in range(16):
            colsum[m] = c40[32 * (m % 2) + m // 2]
        msesum += float(acc[32:40, NP_].sum())

        # cols decode: cell [m, off] <-> b = 512m + off
        colsc = colsum.reshape(-1).copy()
        colsc[:128] = 0.0                          # b < 128: never covered
        nact = np.zeros(N)
        for (k, eng, plo, phi) in PIECES:
            if eng == "A":
                nact[plo:phi] += 64.0              # sign tiles wrote X - 0.5
        colsc[128:] += nact[128:]
        cols += colsc

        for k in range(8):
            r = R[k]
            w = WSTART[k]
            apos = 128 * r + np.arange(128)
            qa = qd[apos]
            radd = np.zeros(128)
            for pi, (kk, eng, plo, phi) in enumerate(PIECES):
                if kk != k:
                    continue
                if eng == "V":
                    radd += acc[:, pi]
                else:
                    radd += (acc[:, pi] + (phi - plo)) / 2.0
            # pollution: device also counted b with pos(b) <= pos(a)
            hi = 128 * (r + 1)
            if hi > w:
                win = np.arange(w, hi)
                qb = qd[win]
                lt = (qb[None, :] < qa[:, None] + MARGIN)
                eq = (qb[None, :] == qa[:, None] + MARGIN)
                posmask = (win[None, :] <= apos[:, None])
                actseg = np.zeros(hi - w, dtype=bool)
                for (kk, eng, plo, phi) in PIECES:
                    if kk == k and eng == "A":
                        lo_i, hi_i = max(plo - w, 0), min(phi, hi) - w
                        if hi_i > lo_i:
                            actseg[lo_i:hi_i] = True
                dveseg = ~actseg
                pv = (lt & posmask & dveseg[None, :]).sum(1)
                pa = ((lt & posmask & actseg[None, :]).sum(1)
                      + 0.5 * (eq & posmask & actseg[None, :]).sum(1))
                radd = radd - pv - pa
                cv = (lt & posmask & dveseg[None, :]).sum(0)
                ca = ((lt & posmask & actseg[None, :]).sum(0)
                      + 0.5 * (eq & posmask & actseg[None, :]).sum(0))
                np.add.at(cols, win, -(cv + ca))
            rows[apos] += radd

    # host-exact diagonal tiles {8k} (not covered by any window)
    for t in range(0, 64, 8):
        qa = qd[128 * t:128 * (t + 1)]
        X = (qa[None, :] < qa[:, None] + MARGIN)
        X &= np.triu(np.ones((128, 128), dtype=bool), k=1)
        rows[128 * t:128 * (t + 1)] += X.sum(1)
        cols[128 * t:128 * (t + 1)] += X.sum(0)

    grad = rows - cols
    TOT = rows.sum()
    ranking = MARGIN * TOT + qtrue @ grad
    g2 = np.sqrt((grad * grad).sum())
    mse = msesum / N
    g1 = 2.0 * np.sqrt(msesum) / N
    return np.float32(mse + g1 / (g2 + EPS) * ranking)


# ------------------------------------------------- numpy device model ----
def _sim_outputs(preds, labels):
    """Produce the same outputs the device would (for offline validation)."""
    preds = np.asarray(preds, dtype=np.float32)
    labels = np.asarray(labels, dtype=np.float32)
    _, qd, _ = _sorted_q(preds, labels)
    out = []
    for c in range(NCORES):
        R = _core_rowtiles(c)
        acc = np.zeros((128, NP_))
        colsum = np.zeros((16, 512))
        c40 = np.zeros((40, 512))
        for pi, (k, eng, plo, phi) in enumerate(PIECES):
            r = R[k]
            qa = qd[128 * r:128 * (r + 1)]
            if eng == "V":
                X = (qd[None, plo:phi] < qa[:, None] + MARGIN).astype(np.float64)
                acc[:, pi] = X.sum(1)
                wgt, T = 1.0, X
            else:
                sgn = np.sign(qa[:, None] + MARGIN - qd[None, plo:phi])
                acc[:, pi] = sgn.sum(1)
                wgt, T = 0.5, sgn
            for m in range(plo // 512, (phi + 511) // 512):
                lo, hi = max(plo, 512 * m), min(phi, 512 * (m + 1))
                if lo < hi:
                    colsum[m, lo - 512 * m:hi - 512 * m] += \
                        wgt * T[:, lo - plo:hi - plo].sum(0)
        rows = slice(c * RPC, (c + 1) * RPC)
        d = (preds[rows] - labels[rows]).astype(np.float32).astype(np.float64)
        msesq = d.reshape(8, 128)
        accT = np.zeros((32, 128), dtype=np.float32)
        accT[:NP_] = acc.T.astype(np.float32)
        accT[NP_, 32:40] = (msesq * msesq).sum(1).astype(np.float32)
        for m in range(16):
            c40[32 * (m % 2) + m // 2] = colsum[m]
        out.append({
            "accT": accT,
            "colsum": c40.astype(np.float32),
        })
    return out


# ------------------------------------------------------------- driver ----
def kernel(preds, labels):
    global LAST_RESULTS
    from concourse.bass_utils import run_bass_kernel_spmd

    if "nc" not in _CACHE:
        _CACHE["nc"] = build_nc()
    in_maps = make_in_maps(preds, labels)
    res = run_bass_kernel_spmd(_CACHE["nc"], in_maps, list(range(NCORES)))
    LAST_RESULTS = res
    return combine(res.results, preds, labels)


# revision 48
# speedup vs baseline: 1.0113x; 1.0113x over previous
"""Trainium2 Bass kernel for nn_BatchRankingMSE_Loss (N=8192, 8 cores).

Reformulation: sort by labels on host (a pure data permutation). With q =
label-sorted preds quantized to fp8e4 (borderline flips land on near-zero
relu terms; g2 shifts ~0.1% -- far inside the 2e-2 gate), define for every
pair a<b (sorted positions) X(a,b) = 1{q_b < q_a + M}. Then
  ranking          = M*TOT + sum_a qtrue_a*rows_a - sum_b qtrue_b*cols_b
  grad_a (ranking) = rows_a - cols_a,   TOT = sum(X)
with rows/cols the row/column sums of X (qtrue = unquantized sorted preds).

Per core (SPMD; core c owns row-tiles R_c = {8k + (c+k)%8}): 8 flip-layout
ops, op k = [128 partitions = a-values of tile R_c[k]] x [free b in
[128*(8k+1), 8192)].  Window segments are split between the DVE
(tensor_scalar is_lt, fused accum -> row sums) and ACT (Sign activation,
fused accum), and every produced X tile streams through the PE with
one-hot stationary columns into a [16,512] PSUM tile -> column sums
(chunk m of 512 b's accumulates into PSUM row m; sign tiles use
0.5-valued stationary).  v2 vs the 46us baseline:
  - qj is fp8 (1 MB not 2), DMA'd via 3 queues (sync/scalar/gpsimd) in 7
    chunks with a tiny first chunk so compute starts ~1.5us in.
  - the one-hot stationary table is generated on device by strided
    memsets (the 139KB broadcast DMA that gated everything is gone).
  - PSUM->SBUF colsum copy runs on the (otherwise idle) Pool engine.
  - fewer, larger pieces; per-engine shares balanced against measured
    rates (DVE 1.042 ns/col, ACT 0.833 ns/col + per-piece overheads).
Window overshoot (b at/below own position) and the 8 uncovered diagonal
tiles {8k} are corrected exactly on host; mse partials also on device.
"""

import numpy as np

MARGIN = 2.0
EPS = 1e-4
N = 8192
NCORES = 8
RPC = N // NCORES        # rows per core = 1024

_CACHE = {}
LAST_RESULTS = None      # test.py introspects timing from here


# ---------------------------------------------------------------- plan ----
def _core_rowtiles(c):
    return [8 * k + (c + k) % 8 for k in range(8)]


WSTART = [128 * (8 * k + 1) for k in range(8)]     # flip-op window starts

# qj DMA chunks: (lo, hi, queue) in per-queue post order. queues: S=sync,
# A=scalar, G=gpsimd.  A [128,x] chunk costs ~128 packets (~1.2us) on a
# queue regardless of x, so two fat chunks per queue beat many thin ones.
CHUNKS = [
    (0, 1024, "S"),
    (1024, 2496, "A"),
    (2496, 4096, "G"),
    (4096, 6144, "A"),
    (6144, 8192, "G"),
]

# pieces: (k, eng, lo, hi) in issue order per engine. eng V=DVE, A=ACT.
# Whole ops per engine (one qip column each) + an arrival split on the two
# biggest ops. Shares: DVE 17024 cols @1.042, ACT 18816 @0.833.
PIECES = [
    (0, "V", 128, 512),      # needs C0a only
    (1, "A", 1152, 2496),    # needs C1
    (0, "V", 512, 4096),     # needs C0b,C1,C2
    (1, "A", 2496, 4096),    # needs C2
    (0, "V", 4096, 8192),
    (1, "A", 4096, 8192),
    (3, "A", 3200, 8192),
    (2, "V", 2176, 8192),
    (4, "A", 4224, 8192),
    (5, "V", 5248, 8192),
    (6, "A", 6272, 8192),
    (7, "A", 7296, 8192),    # small last pieces -> short endgame
]
NP_ = len(PIECES)
NWARM = 16


def _sanity():
    cov = {k: [] for k in range(8)}
    for (k, e, lo, hi) in PIECES:
        cov[k].append((lo, hi))
    for k in range(8):
        segs = sorted(cov[k])
        assert segs[0][0] == WSTART[k], (k, segs)
        assert segs[-1][1] == N, (k, segs)
        for a, b in zip(segs, segs[1:]):
            assert a[1] == b[0], (k, segs)


_sanity()


def _mm_chunks():
    """PE chunk-matmuls: (piece idx, chunk m, lo, hi) ordered by estimated
    piece completion so the PE queue never head-of-line blocks."""
    rate = {"V": 1.042, "A": 0.833}
    over = {"V": 0.37, "A": 0.46}
    # rough chunk arrival (us, body-relative)
    arr_t = {0: 4.4, 1: 4.7, 2: 4.9, 3: 6.4, 4: 6.3}
    def col_avail(c):
        for i, (lo, hi, q) in enumerate(CHUNKS):
            if c <= hi:
                return arr_t[i]
        return 6.4
    eng_t = {"V": 4.5, "A": 4.7}
    done = []
    for pi, (k, e, lo, hi) in enumerate(PIECES):
        start = max(eng_t[e], col_avail(hi))
        t = start + over[e] + rate[e] * (hi - lo) / 1000.0
        eng_t[e] = t
        done.append((t, pi))
    done.sort()
    mms = []
    for _, pi in done:
        (k, e, lo, hi) = PIECES[pi]
        for m in range(lo // 512, (hi + 511) // 512):
            a, b = max(lo, 512 * m), min(hi, 512 * (m + 1))
            if a < b:
                mms.append((pi, m, a, b))
    return mms


MMS = _mm_chunks()


# ------------------------------------------------------------- program ----
def build_nc():
    import concourse.bass as bass
    import concourse.mybir as mybir
    from concourse import bacc, tile

    dt = mybir.dt
    Af = mybir.ActivationFunctionType
    Op = mybir.AluOpType

    nc = bacc.Bacc(None)
    qj_in = nc.dram_tensor("qj", [128, N], dt.float8e4, kind="ExternalInput")
    # small [48, 128] f32: rows 0:8 qip^T, rows 32:40 (preds-labels)^T
    # (d-rows at partition 32: partition bases must be 32-aligned)
    sm_in = nc.dram_tensor("small", [48, 128], dt.float32, kind="ExternalInput")
    accT_out = nc.dram_tensor("accT", [32, 128], dt.float32, kind="ExternalOutput")
    cols_out = nc.dram_tensor("colsum", [40, 512], dt.float32, kind="ExternalOutput")

    with tile.TileContext(nc) as tc:
        with (
            tc.tile_pool(name="persist", bufs=1) as pp,
            tc.tile_pool(name="work", bufs=4) as wp,
            tc.tile_pool(name="psum", bufs=1, space="PSUM") as qp,
        ):
            sm = pp.tile([48, 128], dt.float32)
            qj = pp.tile([128, N], dt.float8e4)
            stoh = pp.tile([128, 544], dt.float16)
            acc = pp.tile([128, 32], dt.float32)
            accT = pp.tile([32, 128], dt.float32)
            csb = pp.tile([40, 512], dt.float32)
            sqms = pp.tile([48, 128], dt.float32)
            dumm = pp.tile([128, 16], dt.float16)
            jj = pp.tile([128, 128], dt.int32)
            pid = pp.tile([128, 1], dt.int32)
            pidf = pp.tile([128, 1], dt.float32)
            iden = pp.tile([128, 128], dt.float32)

            psC = qp.tile([128, 512], dt.float32, tag="psc", name="psc")
            psT = qp.tile([32, 128], dt.float32, tag="pst", name="pst")
            psS = qp.tile([128, 16], dt.float32, tag="pss", name="pss")

            # stationary one-hot table via strided memsets (no DMA). Chunk
            # m maps to PE col-strip j=m%2, PSUM rows 32j+r with r=m//2;
            # col-tiled matmuls use full 32-wide stationaries:
            # V-block r: stoh[:, 32r+r]=1 (stride 33 in [0:256));
            # A-block r: stoh[:, 256+33r]=0.5; rest stays 0 (warmup zeros)
            nc.gpsimd.memset(stoh[:], 0.0)
            nc.gpsimd.memset(stoh[:, 0:256:33], 1.0)
            mid = stoh[:, 256:512]
            nc.gpsimd.memset(mid[:, 0:256:33], 0.5)

            # DMA ladder. sync: tiny C0 first, then the small tensor in its
            # natural [32,128] layout (32 fat packets, no transpose DMA);
            # scalar/gpsimd carry big qj chunks in parallel.
            # C0 then sm on sync: the qip transpose's wait is a cumulative
            # DMA-count on this queue and C0 is needed first anyway
            nc.sync.dma_start(qj[:, 0:512], qj_in[:, 0:512])
            nc.sync.dma_start(qj[:, 512:1024], qj_in[:, 512:1024])
            nc.scalar.dma_start(sm[:], sm_in[:])
            nc.scalar.dma_start(qj[:, 1024:2496], qj_in[:, 1024:2496])
            # identity ingredients (iota is gpsimd-only; cheap)
            nc.gpsimd.iota(jj[:], [[1, 128]], base=0, channel_multiplier=0)
            nc.gpsimd.iota(pid[:], [[0, 1]], base=0, channel_multiplier=1)
            nc.gpsimd.dma_start(qj[:, 2496:4096], qj_in[:, 2496:4096])
            nc.scalar.dma_start(qj[:, 4096:6144], qj_in[:, 4096:6144])
            nc.gpsimd.dma_start(qj[:, 6144:8192], qj_in[:, 6144:8192])

            # identity matrix (DVE, idle early): for the qip transpose now
            # and the accT transpose at the end
            nc.vector.tensor_copy(pidf[:], pid[:])
            nc.vector.tensor_scalar(iden[:], jj[:], pidf[:, 0:1], None,
                                    op0=Op.is_equal)

            # PE warmup: zero-stationary matmuls (add 0 into psC) lift the
            # HAM clock gate before real chunk matmuls arrive.
            warm = [0]

            def warmup_mm(n):
                for _ in range(n):
                    j = warm[0] % 2
                    nc.tensor.matmul(psC[32 * j:32 * j + 32, 0:512],
                                     stoh[:, 512:544], stoh[:, 0:512],
                                     start=(warm[0] < 2), stop=False,
                                     tile_position=(0, 32 * j))
                    warm[0] += 1

            # warmups + qip transpose + warmups: a solid block of PE busy
            # (~3.4us) flips the HAM clock gate to 8/8 right as real chunk
            # matmuls arrive, and the transpose lands mid-block.
            warmup_mm(4)
            qipf = pp.tile([128, 8], dt.float32)
            nc.tensor.transpose(psS[:], sm[0:16, :], iden[0:16, 0:16])
            nc.vector.tensor_copy(qipf[:], psS[:, 0:8])
            qip = qipf

            # load the Sign table while DMAs stream (dummy op on stoh)
            nc.scalar.activation(dumm[:], stoh[:, 0:16],
                                 Af.Sign, bias=0.0, scale=1.0)

            warmup_mm(NWARM - 4)

            tiles = {}
            emitted = set()
            last_of_strip = {}
            for mi, (pi, m, lo, hi) in enumerate(MMS):
                last_of_strip[m % 2] = mi

            def emit_piece(pi):
                k, eng, plo, phi = PIECES[pi]
                fd = phi - plo
                t = wp.tile([128, fd], dt.float16, tag=eng)
                if eng == "V":
                    nc.vector.tensor_scalar(
                        t[:], qj[:, plo:phi], qip[:, k:k + 1], 0.0,
                        op0=Op.is_lt, op1=Op.add,
                        accum_out=acc[:, pi:pi + 1])
                else:
                    nc.scalar.activation(
                        t[:], qj[:, plo:phi], Af.Sign, bias=qip[:, k:k + 1],
                        scale=-1.0, accum_out=acc[:, pi:pi + 1])
                tiles[pi] = t

            # first DVE piece, then the (tiny) mse partials in DVE's early
            # idle window: host supplies d=(p-l) rows; device squares and
            # free-sums them into acc[8:16, NP_] (no transpose needed)
            nc.vector.scalar_tensor_tensor(
                sqms[32:40, :], sm[32:40, :], 1.0, sm[32:40, :],
                op0=Op.mult, op1=Op.mult,
                accum_out=acc[32:40, NP_:NP_ + 1])
            emit_piece(0)
            emitted.add(0)

            # issue pieces in engine order; MMs in completion-estimate order;
            # extra warmups bridge the sparse early MM flow (HAM stays fed)
            seen_groups = 0
            last_pi = None
            for mi, (pi, m, lo, hi) in enumerate(MMS):
                if pi not in emitted:
                    eng = PIECES[pi][1]
                    for pj, (k2, e2, l2, h2) in enumerate(PIECES):
                        if e2 == eng and pj not in emitted and pj <= pi:
                            emit_piece(pj)
                            emitted.add(pj)
                if pi != last_pi:
                    if seen_groups < 2:
                        warmup_mm(4)
                    seen_groups += 1
                    last_pi = pi
                k, eng, plo, phi = PIECES[pi]
                t = tiles[pi]
                j, r = m % 2, m // 2
                sv = 32 * r + (256 if eng == "A" else 0)
                nc.tensor.matmul(
                    psC[32 * j:32 * j + 32, lo - 512 * m:hi - 512 * m],
                    stoh[:, sv:sv + 32], t[:, lo - plo:hi - plo],
                    start=False, stop=(mi == last_of_strip[j]),
                    tile_position=(0, 32 * j))

            # outputs: acc (incl mse col) PE-transposed -> [32,128], one
            # packet per row instead of a 128x small-packet storm.
            nc.tensor.transpose(psT[:], acc[:], iden[:])
            nc.vector.tensor_copy(accT[:], psT[:])
            nc.vector.tensor_copy(csb[0:8, :], psC[0:8, :])
            nc.scalar.activation(csb[32:40, :], psC[32:40, :], Af.Copy)
            nc.sync.dma_start(accT_out[:], accT[:])
            nc.scalar.dma_start(cols_out[:], csb[:])
    if not nc.is_finalized():
        nc.finalize()
    return nc


# ---------------------------------------------------------- host side ----
def _f8():
    import ml_dtypes
    return ml_dtypes.float8_e4m3


def _sorted_q(preds, labels):
    labels32 = np.asarray(labels, dtype=np.float32)
    perm = np.argsort(labels32, kind="stable")
    psort = np.asarray(preds, dtype=np.float32)[perm]
    q8 = psort.astype(_f8())
    return q8, q8.astype(np.float64), psort.astype(np.float64)


def make_in_maps(preds, labels):
    preds = np.asarray(preds, dtype=np.float32)
    labels = np.asarray(labels, dtype=np.float32)
    q8, qd, _ = _sorted_q(preds, labels)
    qjrep = np.ascontiguousarray(np.broadcast_to(q8, (128, N)))
    in_maps = []
    for c in range(NCORES):
        R = _core_rowtiles(c)
        i_of_m = np.concatenate([128 * r + np.arange(128) for r in R])
        qip = (qd[i_of_m] + MARGIN).reshape(8, 128)
        rows = slice(c * RPC, (c + 1) * RPC)
        small = np.zeros((48, 128), dtype=np.float32)
        small[0:8] = qip.astype(np.float32)
        small[32:40] = (preds[rows] - labels[rows]).reshape(8, 128)
        in_maps.append({"qj": qjrep, "small": small})
    return in_maps


def combine(results, preds, labels):
    """Fold device partials into the scalar loss (host, f64, exact)."""
    _, qd, qtrue = _sorted_q(preds, labels)

    rows = np.zeros(N)
    cols = np.zeros(N)
    msesum = 0.0
    for c in range(NCORES):
        res = results[c]
        R = _core_rowtiles(c)
        acc = res["accT"].astype(np.float64).T          # [128, 32]
        c40 = res["colsum"].astype(np.float64)          # [40, 512]
        colsum = np.zeros((16, 512))
        for m in range(16):
            colsum[m] = c40[32 * (m % 2) + m // 2]
        msesum += float(acc[32:40, NP_].sum())

        # cols decode: cell [m, off] <-> b = 512m + off
        colsc = colsum.reshape(-1).copy()
        colsc[:128] = 0.0                          # b < 128: never covered
        nact = np.zeros(N)
        for (k, eng, plo, phi) in PIECES:
            if eng == "A":
                nact[plo:phi] += 64.0              # sign tiles wrote X - 0.5
        colsc[128:] += nact[128:]
        cols += colsc

        for k in range(8):
            r = R[k]
            w = WSTART[k]
            apos = 128 * r + np.arange(128)
            qa = qd[apos]
            radd = np.zeros(128)
            for pi, (kk, eng, plo, phi) in enumerate(PIECES):
                if kk != k:
                    continue
                if eng == "V":
                    radd += acc[:, pi]
                else:
                    radd += (acc[:, pi] + (phi - plo)) / 2.0
            # pollution: device also counted b with pos(b) <= pos(a)
            hi = 128 * (r + 1)
            if hi > w:
                win = np.arange(w, hi)
                qb = qd[win]
                lt = (qb[None, :] < qa[:, None] + MARGIN)
                eq = (qb[None, :] == qa[:, None] + MARGIN)
                posmask = (win[None, :] <= apos[:, None])
                actseg = np.zeros(hi - w, dtype=bool)
                for (kk, eng, plo, phi) in PIECES:
                    if kk == k and eng == "A":
                        lo_i, hi_i = max(plo - w, 0), min(phi, hi) - w
                        if hi_i > lo_i:
                            actseg[lo_i:hi_i] = True
                dveseg = ~actseg
                pv = (lt & posmask & dveseg[None, :]).sum(1)
                pa = ((lt & posmask & actseg[None, :]).sum(1)
                      + 0.5 * (eq & posmask & actseg[None, :]).sum(1))
                radd = radd - pv - pa
                cv = (lt & posmask & dveseg[None, :]).sum(0)
                ca = ((lt & posmask & actseg[None, :]).sum(0)
                      + 0.5 * (eq & posmask & actseg[None, :]).sum(0))
                np.add.at(cols, win, -(cv + ca))
            rows[apos] += radd

    # host-exact diagonal tiles {8k} (not covered by any window)
    for t in range(0, 64, 8):
        qa = qd[128 * t:128 * (t + 1)]
        X = (qa[None, :] < qa[:, None] + MARGIN)
        X &= np.triu(np.ones((128, 128), dtype=bool), k=1)
        rows[128 * t:128 * (t + 1)] += X.sum(1)
        cols[128 * t:128 * (t + 1)] += X.sum(0)

    grad = rows - cols
    TOT = rows.sum()
    ranking = MARGIN * TOT + qtrue @ grad
    g2 = np.sqrt((grad * grad).sum())
    mse = msesum / N
    g1 = 2.0 * np.sqrt(msesum) / N
    return np.float32(mse + g1 / (g2 + EPS) * ranking)


# ------------------------------------------------- numpy device model ----
def _sim_outputs(preds, labels):
    """Produce the same outputs the device would (for offline validation)."""
    preds = np.asarray(preds, dtype=np.float32)
    labels = np.asarray(labels, dtype=np.float32)
    _, qd, _ = _sorted_q(preds, labels)
    out = []
    for c in range(NCORES):
        R = _core_rowtiles(c)
        acc = np.zeros((128, NP_))
        colsum = np.zeros((16, 512))
        c40 = np.zeros((40, 512))
        for pi, (k, eng, plo, phi) in enumerate(PIECES):
            r = R[k]
            qa = qd[128 * r:128 * (r + 1)]
            if eng == "V":
                X = (qd[None, plo:phi] < qa[:, None] + MARGIN).astype(np.float64)
                acc[:, pi] = X.sum(1)
                wgt, T = 1.0, X
            else:
                sgn = np.sign(qa[:, None] + MARGIN - qd[None, plo:phi])
                acc[:, pi] = sgn.sum(1)
                wgt, T = 0.5, sgn
            for m in range(plo // 512, (phi + 511) // 512):
                lo, hi = max(plo, 512 * m), min(phi, 512 * (m + 1))
                if lo < hi:
                    colsum[m, lo - 512 * m:hi - 512 * m] += \
                        wgt * T[:, lo - plo:hi - plo].sum(0)
        rows = slice(c * RPC, (c + 1) * RPC)
        d = (preds[rows] - labels[rows]).astype(np.float32).astype(np.float64)
        msesq = d.reshape(8, 128)
        accT = np.zeros((32, 128), dtype=np.float32)
        accT[:NP_] = acc.T.astype(np.float32)
        accT[NP_, 32:40] = (msesq * msesq).sum(1).astype(np.float32)
        for m in range(16):
            c40[32 * (m % 2) + m // 2] = colsum[m]
        out.append({
            "accT": accT,
            "colsum": c40.astype(np.float32),
        })
    return out


# ------------------------------------------------------------- driver ----
def kernel(preds, labels):
    global LAST_RESULTS
    from concourse.bass_utils import run_bass_kernel_spmd

    if "nc" not in _CACHE:
        _CACHE["nc"] = build_nc()
    in_maps = make_in_maps(preds, labels)
    res = run_bass_kernel_spmd(_CACHE["nc"], in_maps, list(range(NCORES)))
    LAST_RESULTS = res
    return combine(res.results, preds, labels)


# revision 49
# speedup vs baseline: 1.0236x; 1.0122x over previous
"""Trainium2 Bass kernel for nn_BatchRankingMSE_Loss (N=8192, 8 cores).

Reformulation: sort by labels on host (a pure data permutation). With q =
label-sorted preds quantized to fp8e4 (borderline flips land on near-zero
relu terms; g2 shifts ~0.1% -- far inside the 2e-2 gate), define for every
pair a<b (sorted positions) X(a,b) = 1{q_b < q_a + M}. Then
  ranking          = M*TOT + sum_a qtrue_a*rows_a - sum_b qtrue_b*cols_b
  grad_a (ranking) = rows_a - cols_a,   TOT = sum(X)
with rows/cols the row/column sums of X (qtrue = unquantized sorted preds).

Per core (SPMD; core c owns row-tiles R_c = {8k + (c+k)%8}): 8 flip-layout
ops, op k = [128 partitions = a-values of tile R_c[k]] x [free b in
[128*(8k+1), 8192)].  Window segments are split between the DVE
(tensor_scalar is_lt, fused accum -> row sums, 1.042 ns/col) and ACT
(Sign activation, fused accum, 0.833 ns/col); every produced X tile also
streams through the PE with one-hot stationaries into PSUM -> column
sums.  Performance structure (vs the 46us baseline):
  - qj is fp8 (1 MB, not 2); 6 chunks over the 3 DMA-capable queues
    (sync/scalar/gpsimd) in parallel; a small first chunk lets the first
    DVE piece start ~err4.5us in. A [128,x] chunk costs ~128 packets
    (~1.3us+) per queue regardless of x.
  - all stationary tables are built on device by strided memsets; the
    small input (qip rows, (p-l) rows) arrives as one [48,128] f32 DMA
    and qip is transposed onto 128 partitions by a PE transpose-matmul
    (identity built from gpsimd iotas + a DVE is_equal).
  - PE column sums are COLUMN-TILED: chunk m of 512 b's runs on PE
    col-strip j=m%2 via tile_position=(0,32j) with a 32-wide one-hot
    (hot at r=m//2, value 1.0 for DVE tiles / 0.5 for sign tiles) into
    PSUM rows 32j+r of a [128,512] bank -> two concurrent MM streams
    (~2x PE throughput, halves the cold-clock penalty too).
  - a solid block of zero-stationary warmup matmuls (NWARM, alternating
    strips) + early bridge warmups keep the PE HAM clock gate at 8/8.
  - mse partials: host sends d=(p-l); the device squares/free-sums into
    acc[32:40] (no transpose needed; partition bases must be 32-aligned).
  - outputs: acc (rowsums + mse col) is PE-transposed to [32,128] and
    DMA'd as 32 fat packets (f32 [128,x] outputs would be a 128-packet
    storm); colsum strips are copied PSUM->SBUF on DVE+ACT in parallel.
Window overshoot (b at/below own position) and the 8 uncovered diagonal
tiles {8k} are corrected exactly on host; the scalar loss is assembled
on host in f64 from exact integer-ish device partials.
"""

import numpy as np

MARGIN = 2.0
EPS = 1e-4
N = 8192
NCORES = 8
RPC = N // NCORES        # rows per core = 1024

_CACHE = {}
LAST_RESULTS = None      # test.py introspects timing from here


# ---------------------------------------------------------------- plan ----
def _core_rowtiles(c):
    return [8 * k + (c + k) % 8 for k in range(8)]


WSTART = [128 * (8 * k + 1) for k in range(8)]     # flip-op window starts

# qj DMA chunks: (lo, hi, queue) in per-queue post order. queues: S=sync,
# A=scalar, G=gpsimd.  A [128,x] chunk costs ~128 packets (~1.2us) on a
# queue regardless of x, so two fat chunks per queue beat many thin ones.
CHUNKS = [
    (0, 1024, "S"),
    (1024, 2496, "A"),
    (2496, 4096, "G"),
    (4096, 6144, "A"),
    (6144, 8192, "G"),
]

# pieces: (k, eng, lo, hi) in issue order per engine. eng V=DVE, A=ACT.
# Whole ops per engine (one qip column each) + an arrival split on the two
# biggest ops. Shares: DVE 17024 cols @1.042, ACT 18816 @0.833.
PIECES = [
    (0, "V", 128, 512),      # needs C0a only
    (1, "A", 1152, 2496),    # needs C1
    (0, "V", 512, 4096),     # needs C0b,C1,C2
    (1, "A", 2496, 4096),    # needs C2
    (0, "V", 4096, 8192),
    (1, "A", 4096, 8192),
    (3, "A", 3200, 8192),
    (2, "V", 2176, 8192),
    (4, "A", 4224, 8192),
    (5, "V", 5248, 8192),
    (6, "A", 6272, 8192),
    (7, "A", 7296, 8192),    # small last pieces -> short endgame
]
NP_ = len(PIECES)
NWARM = 16


def _sanity():
    cov = {k: [] for k in range(8)}
    for (k, e, lo, hi) in PIECES:
        cov[k].append((lo, hi))
    for k in range(8):
        segs = sorted(cov[k])
        assert segs[0][0] == WSTART[k], (k, segs)
        assert segs[-1][1] == N, (k, segs)
        for a, b in zip(segs, segs[1:]):
            assert a[1] == b[0], (k, segs)


_sanity()


def _mm_chunks():
    """PE chunk-matmuls: (piece idx, chunk m, lo, hi) ordered by estimated
    piece completion so the PE queue never head-of-line blocks."""
    rate = {"V": 1.042, "A": 0.833}
    over = {"V": 0.37, "A": 0.46}
    # rough chunk arrival (us, body-relative)
    arr_t = {0: 4.4, 1: 4.7, 2: 4.9, 3: 6.4, 4: 6.3}
    def col_avail(c):
        for i, (lo, hi, q) in enumerate(CHUNKS):
            if c <= hi:
                return arr_t[i]
        return 6.4
    eng_t = {"V": 4.5, "A": 4.7}
    done = []
    for pi, (k, e, lo, hi) in enumerate(PIECES):
        start = max(eng_t[e], col_avail(hi))
        t = start + over[e] + rate[e] * (hi - lo) / 1000.0
        eng_t[e] = t
        done.append((t, pi))
    done.sort()
    mms = []
    for _, pi in done:
        (k, e, lo, hi) = PIECES[pi]
        for m in range(lo // 512, (hi + 511) // 512):
            a, b = max(lo, 512 * m), min(hi, 512 * (m + 1))
            if a < b:
                mms.append((pi, m, a, b))
    return mms


MMS = _mm_chunks()


# ------------------------------------------------------------- program ----
def build_nc():
    import concourse.bass as bass
    import concourse.mybir as mybir
    from concourse import bacc, tile

    dt = mybir.dt
    Af = mybir.ActivationFunctionType
    Op = mybir.AluOpType

    nc = bacc.Bacc(None)
    qj_in = nc.dram_tensor("qj", [128, N], dt.float8e4, kind="ExternalInput")
    # small [48, 128] f32: rows 0:8 qip^T, rows 32:40 (preds-labels)^T
    # (d-rows at partition 32: partition bases must be 32-aligned)
    sm_in = nc.dram_tensor("small", [48, 128], dt.float32, kind="ExternalInput")
    accT_out = nc.dram_tensor("accT", [32, 128], dt.float32, kind="ExternalOutput")
    cols_out = nc.dram_tensor("colsum", [40, 512], dt.float32, kind="ExternalOutput")

    with tile.TileContext(nc) as tc:
        with (
            tc.tile_pool(name="persist", bufs=1) as pp,
            tc.tile_pool(name="work", bufs=4) as wp,
            tc.tile_pool(name="psum", bufs=1, space="PSUM") as qp,
        ):
            sm = pp.tile([48, 128], dt.float32)
            qj = pp.tile([128, N], dt.float8e4)
            stoh = pp.tile([128, 544], dt.float16)
            acc = pp.tile([128, 32], dt.float32)
            accT = pp.tile([32, 128], dt.float32)
            csb = pp.tile([40, 512], dt.float32)
            sqms = pp.tile([48, 128], dt.float32)
            dumm = pp.tile([128, 16], dt.float16)
            jj = pp.tile([128, 128], dt.int32)
            pid = pp.tile([128, 1], dt.int32)
            pidf = pp.tile([128, 1], dt.float32)
            iden = pp.tile([128, 128], dt.float32)

            psC = qp.tile([128, 512], dt.float32, tag="psc", name="psc")
            psT = qp.tile([32, 128], dt.float32, tag="pst", name="pst")
            psS = qp.tile([128, 16], dt.float32, tag="pss", name="pss")

            # stationary one-hot table via strided memsets (no DMA). Chunk
            # m maps to PE col-strip j=m%2, PSUM rows 32j+r with r=m//2;
            # col-tiled matmuls use full 32-wide stationaries:
            # V-block r: stoh[:, 32r+r]=1 (stride 33 in [0:256));
            # A-block r: stoh[:, 256+33r]=0.5; rest stays 0 (warmup zeros)
            nc.gpsimd.memset(stoh[:], 0.0)
            nc.gpsimd.memset(stoh[:, 0:256:33], 1.0)
            mid = stoh[:, 256:512]
            nc.gpsimd.memset(mid[:, 0:256:33], 0.5)

            # DMA ladder. sync: tiny C0 first, then the small tensor in its
            # natural [32,128] layout (32 fat packets, no transpose DMA);
            # scalar/gpsimd carry big qj chunks in parallel.
            # C0 then sm on sync: the qip transpose's wait is a cumulative
            # DMA-count on this queue and C0 is needed first anyway
            nc.sync.dma_start(qj[:, 0:512], qj_in[:, 0:512])
            nc.sync.dma_start(qj[:, 512:1024], qj_in[:, 512:1024])
            nc.scalar.dma_start(sm[:], sm_in[:])
            nc.scalar.dma_start(qj[:, 1024:2496], qj_in[:, 1024:2496])
            # identity ingredients (iota is gpsimd-only; cheap)
            nc.gpsimd.iota(jj[:], [[1, 128]], base=0, channel_multiplier=0)
            nc.gpsimd.iota(pid[:], [[0, 1]], base=0, channel_multiplier=1)
            nc.gpsimd.dma_start(qj[:, 2496:4096], qj_in[:, 2496:4096])
            nc.scalar.dma_start(qj[:, 4096:6144], qj_in[:, 4096:6144])
            nc.gpsimd.dma_start(qj[:, 6144:8192], qj_in[:, 6144:8192])

            # identity matrix (DVE, idle early): for the qip transpose now
            # and the accT transpose at the end
            nc.vector.tensor_copy(pidf[:], pid[:])
            nc.vector.tensor_scalar(iden[:], jj[:], pidf[:, 0:1], None,
                                    op0=Op.is_equal)

            # PE warmup: zero-stationary matmuls (add 0 into psC) lift the
            # HAM clock gate before real chunk matmuls arrive.
            warm = [0]

            def warmup_mm(n):
                for _ in range(n):
                    j = warm[0] % 2
                    nc.tensor.matmul(psC[32 * j:32 * j + 32, 0:512],
                                     stoh[:, 512:544], stoh[:, 0:512],
                                     start=(warm[0] < 2), stop=False,
                                     tile_position=(0, 32 * j))
                    warm[0] += 1

            # warmups + qip transpose + warmups: a solid block of PE busy
            # (~3.4us) flips the HAM clock gate to 8/8 right as real chunk
            # matmuls arrive, and the transpose lands mid-block.
            warmup_mm(4)
            qipf = pp.tile([128, 8], dt.float32)
            nc.tensor.transpose(psS[:], sm[0:16, :], iden[0:16, 0:16])
            nc.vector.tensor_copy(qipf[:], psS[:, 0:8])
            qip = qipf

            # load the Sign table while DMAs stream (dummy op on stoh)
            nc.scalar.activation(dumm[:], stoh[:, 0:16],
                                 Af.Sign, bias=0.0, scale=1.0)

            warmup_mm(NWARM - 4)

            tiles = {}
            emitted = set()
            last_of_strip = {}
            for mi, (pi, m, lo, hi) in enumerate(MMS):
                last_of_strip[m % 2] = mi

            def emit_piece(pi):
                k, eng, plo, phi = PIECES[pi]
                fd = phi - plo
                t = wp.tile([128, fd], dt.float16, tag=eng)
                if eng == "V":
                    nc.vector.tensor_scalar(
                        t[:], qj[:, plo:phi], qip[:, k:k + 1], 0.0,
                        op0=Op.is_lt, op1=Op.add,
                        accum_out=acc[:, pi:pi + 1])
                else:
                    nc.scalar.activation(
                        t[:], qj[:, plo:phi], Af.Sign, bias=qip[:, k:k + 1],
                        scale=-1.0, accum_out=acc[:, pi:pi + 1])
                tiles[pi] = t

            # first DVE piece, then the (tiny) mse partials in DVE's early
            # idle window: host supplies d=(p-l) rows; device squares and
            # free-sums them into acc[8:16, NP_] (no transpose needed)
            nc.vector.scalar_tensor_tensor(
                sqms[32:40, :], sm[32:40, :], 1.0, sm[32:40, :],
                op0=Op.mult, op1=Op.mult,
                accum_out=acc[32:40, NP_:NP_ + 1])
            emit_piece(0)
            emitted.add(0)

            # issue pieces in engine order; MMs in completion-estimate order;
            # extra warmups bridge the sparse early MM flow (HAM stays fed)
            seen_groups = 0
            last_pi = None
            for mi, (pi, m, lo, hi) in enumerate(MMS):
                if pi not in emitted:
                    eng = PIECES[pi][1]
                    for pj, (k2, e2, l2, h2) in enumerate(PIECES):
                        if e2 == eng and pj not in emitted and pj <= pi:
                            emit_piece(pj)
                            emitted.add(pj)
                if pi != last_pi:
                    if seen_groups < 2:
                        warmup_mm(4)
                    seen_groups += 1
                    last_pi = pi
                k, eng, plo, phi = PIECES[pi]
                t = tiles[pi]
                j, r = m % 2, m // 2
                sv = 32 * r + (256 if eng == "A" else 0)
                nc.tensor.matmul(
                    psC[32 * j:32 * j + 32, lo - 512 * m:hi - 512 * m],
                    stoh[:, sv:sv + 32], t[:, lo - plo:hi - plo],
                    start=False, stop=(mi == last_of_strip[j]),
                    tile_position=(0, 32 * j))

            # outputs: acc (incl mse col) PE-transposed -> [32,128], one
            # packet per row instead of a 128x small-packet storm.
            nc.tensor.transpose(psT[:], acc[:], iden[:])
            nc.vector.tensor_copy(accT[:], psT[:])
            nc.vector.tensor_copy(csb[0:8, :], psC[0:8, :])
            nc.scalar.activation(csb[32:40, :], psC[32:40, :], Af.Copy)
            nc.sync.dma_start(accT_out[:], accT[:])
            nc.scalar.dma_start(cols_out[:], csb[:])
    if not nc.is_finalized():
        nc.finalize()
    return nc


# ---------------------------------------------------------- host side ----
def _f8():
    import ml_dtypes
    return ml_dtypes.float8_e4m3


def _sorted_q(preds, labels):
    labels32 = np.asarray(labels, dtype=np.float32)
    perm = np.argsort(labels32, kind="stable")
    psort = np.asarray(preds, dtype=np.float32)[perm]
    q8 = psort.astype(_f8())
    return q8, q8.astype(np.float64), psort.astype(np.float64)


def make_in_maps(preds, labels):
    preds = np.asarray(preds, dtype=np.float32)
    labels = np.asarray(labels, dtype=np.float32)
    q8, qd, _ = _sorted_q(preds, labels)
    qjrep = np.ascontiguousarray(np.broadcast_to(q8, (128, N)))
    in_maps = []
    for c in range(NCORES):
        R = _core_rowtiles(c)
        i_of_m = np.concatenate([128 * r + np.arange(128) for r in R])
        qip = (qd[i_of_m] + MARGIN).reshape(8, 128)
        rows = slice(c * RPC, (c + 1) * RPC)
        small = np.zeros((48, 128), dtype=np.float32)
        small[0:8] = qip.astype(np.float32)
        small[32:40] = (preds[rows] - labels[rows]).reshape(8, 128)
        in_maps.append({"qj": qjrep, "small": small})
    return in_maps


def combine(results, preds, labels):
    """Fold device partials into the scalar loss (host, f64, exact)."""
    _, qd, qtrue = _sorted_q(preds, labels)

    rows = np.zeros(N)
    cols = np.zeros(N)
    msesum = 0.0
    for c in range(NCORES):
        res = results[c]
        R = _core_rowtiles(c)
        acc = res["accT"].astype(np.float64).T          # [128, 32]
        c40 = res["colsum"].astype(np.float64)          # [40, 512]
        colsum = np.zeros((16, 512))
        for m in range(16):
            colsum[m] = c40[32 * (m % 2) + m // 2]
        msesum += float(acc[32:40, NP_].sum())

        # cols decode: cell [m, off] <-> b = 512m + off
        colsc = colsum.reshape(-1).copy()
        colsc[:128] = 0.0                          # b < 128: never covered
        nact = np.zeros(N)
        for (k, eng, plo, phi) in PIECES:
            if eng == "A":
                nact[plo:phi] += 64.0              # sign tiles wrote X - 0.5
        colsc[128:] += nact[128:]
        cols += colsc

        for k in range(8):
            r = R[k]
            w = WSTART[k]
            apos = 128 * r + np.arange(128)
            qa = qd[apos]
            radd = np.zeros(128)
            for pi, (kk, eng, plo, phi) in enumerate(PIECES):
                if kk != k:
                    continue
                if eng == "V":
                    radd += acc[:, pi]
                else:
                    radd += (acc[:, pi] + (phi - plo)) / 2.0
            # pollution: device also counted b with pos(b) <= pos(a)
            hi = 128 * (r + 1)
            if hi > w:
                win = np.arange(w, hi)
                qb = qd[win]
                lt = (qb[None, :] < qa[:, None] + MARGIN)
                eq = (qb[None, :] == qa[:, None] + MARGIN)
                posmask = (win[None, :] <= apos[:, None])
                actseg = np.zeros(hi - w, dtype=bool)
                for (kk, eng, plo, phi) in PIECES:
                    if kk == k and eng == "A":
                        lo_i, hi_i = max(plo - w, 0), min(phi, hi) - w
                        if hi_i > lo_i:
                            actseg[lo_i:hi_i] = True
                dveseg = ~actseg
                pv = (lt & posmask & dveseg[None, :]).sum(1)
                pa = ((lt & posmask & actseg[None, :]).sum(1)
                      + 0.5 * (eq & posmask & actseg[None, :]).sum(1))
                radd = radd - pv - pa
                cv = (lt & posmask & dveseg[None, :]).sum(0)
                ca = ((lt & posmask & actseg[None, :]).sum(0)
                      + 0.5 * (eq & posmask & actseg[None, :]).sum(0))
                np.add.at(cols, win, -(cv + ca))
            rows[apos] += radd

    # host-exact diagonal tiles {8k} (not covered by any window)
    for t in range(0, 64, 8):
        qa = qd[128 * t:128 * (t + 1)]
        X = (qa[None, :] < qa[:, None] + MARGIN)
        X &= np.triu(np.ones((128, 128), dtype=bool), k=1)
        rows[128 * t:128 * (t + 1)] += X.sum(1)
        cols[128 * t:128 * (t + 1)] += X.sum(0)

    grad = rows - cols
    TOT = rows.sum()
    ranking = MARGIN * TOT + qtrue @ grad
    g2 = np.sqrt((grad * grad).sum())
    mse = msesum / N
    g1 = 2.0 * np.sqrt(msesum) / N
    return np.float32(mse + g1 / (g2 + EPS) * ranking)


# ------------------------------------------------- numpy device model ----
def _sim_outputs(preds, labels):
    """Produce the same outputs the device would (for offline validation)."""
    preds = np.asarray(preds, dtype=np.float32)
    labels = np.asarray(labels, dtype=np.float32)
    _, qd, _ = _sorted_q(preds, labels)
    out = []
    for c in range(NCORES):
        R = _core_rowtiles(c)
        acc = np.zeros((128, NP_))
        colsum = np.zeros((16, 512))
        c40 = np.zeros((40, 512))
        for pi, (k, eng, plo, phi) in enumerate(PIECES):
            r = R[k]
            qa = qd[128 * r:128 * (r + 1)]
            if eng == "V":
                X = (qd[None, plo:phi] < qa[:, None] + MARGIN).astype(np.float64)
                acc[:, pi] = X.sum(1)
                wgt, T = 1.0, X
            else:
                sgn = np.sign(qa[:, None] + MARGIN - qd[None, plo:phi])
                acc[:, pi] = sgn.sum(1)
                wgt, T = 0.5, sgn
            for m in range(plo // 512, (phi + 511) // 512):
                lo, hi = max(plo, 512 * m), min(phi, 512 * (m + 1))
                if lo < hi:
                    colsum[m, lo - 512 * m:hi - 512 * m] += \
                        wgt * T[:, lo - plo:hi - plo].sum(0)
        rows = slice(c * RPC, (c + 1) * RPC)
        d = (preds[rows] - labels[rows]).astype(np.float32).astype(np.float64)
        msesq = d.reshape(8, 128)
        accT = np.zeros((32, 128), dtype=np.float32)
        accT[:NP_] = acc.T.astype(np.float32)
        accT[NP_, 32:40] = (msesq * msesq).sum(1).astype(np.float32)
        for m in range(16):
            c40[32 * (m % 2) + m // 2] = colsum[m]
        out.append({
            "accT": accT,
            "colsum": c40.astype(np.float32),
        })
    return out


# ------------------------------------------------------------- driver ----
def kernel(preds, labels):
    global LAST_RESULTS
    from concourse.bass_utils import run_bass_kernel_spmd

    if "nc" not in _CACHE:
        _CACHE["nc"] = build_nc()
    in_maps = make_in_maps(preds, labels)
    res = run_bass_kernel_spmd(_CACHE["nc"], in_maps, list(range(NCORES)))
    LAST_RESULTS = res
    return combine(res.results, preds, labels)
